# revision 20
# baseline (speedup 1.0000x reference)
"""Trainium2 Bass kernel for single-head attention layer.

Problem: B=4, S=2048, H=1024 fp32.
  q = x @ Wq.T + bq ; k = x @ Wk.T + bk ; v = x @ Wv.T + bv
  out = softmax(q @ k.T / sqrt(H)) @ v

Sharding (8 cores): core c handles batch b=c//2 and key-half half=c%2.
Each core computes (for its batch):
  qT   [H, 2048]  = scaled Q projection for all queries (duplicated per pair)
  kT   [H, 1024]  = keys for its half
  V    [1024, H]  = values for its half
  E    [1024k, 2048q] = exp(scores^T)  (no max subtraction -- scores ~ N(0,1),
                        exp is safe in fp32)
  U    [2048, H]  = E.T @ V   (unnormalized output, fp32 PSUM accum)
  l    [2048]     = per-query sum of E (ones-column matmul)
Host combines: out[b] = (U0+U1) / (l0+l1)[:, None].

All host-side prep (transposes, 1/sqrt(H) folding into Wq/bq, bf16 casts)
is free -- only NEFF execution time counts.
"""

import numpy as np
import ml_dtypes

import concourse.bass as bass
import concourse.mybir as mybir
import concourse.tile as tile
from concourse import bacc
from concourse.bass_utils import run_bass_kernel_spmd

BF16 = mybir.dt.bfloat16
F32 = mybir.dt.float32

B, S, H = 4, 2048, 1024
SH = S // 2          # per-core key half
P = 128
HT = H // P          # 8 h-tiles (contraction for projections)
OT = H // P          # 8 o-tiles
QC = S // 512        # 4 q-chunks of 512
KC = SH // 512       # 2 k-chunks of 512
OC = H // 512        # 2 o-chunks of 512
MT = SH // P         # 8 key tiles (my half)
IT = S // P          # 16 query tiles

Act = mybir.ActivationFunctionType


def build_nc(clone=False, loop_n=None):
    """clone=True: no external inputs (memset instead), u internal -- for timing.
    loop_n: wrap the body in a hardware For_i loop (timing amplification)."""
    nc = bacc.Bacc("TRN2", target_bir_lowering=False, debug=False, num_devices=1)

    if not clone:
        xt = nc.dram_tensor("xt", [H, S], BF16, kind="ExternalInput")    # x[b].T
        xth = nc.dram_tensor("xth", [H, SH], BF16, kind="ExternalInput")
        wqt = nc.dram_tensor("wqt", [H, H], BF16, kind="ExternalInput")  # Wq.T/32
        wkt = nc.dram_tensor("wkt", [H, H], BF16, kind="ExternalInput")  # Wk.T
        wvt = nc.dram_tensor("wvt", [H, H], BF16, kind="ExternalInput")  # Wv.T
        bqs = nc.dram_tensor("bqs", [H], F32, kind="ExternalInput")      # bq/32
        bk = nc.dram_tensor("bk", [H], F32, kind="ExternalInput")
        bv = nc.dram_tensor("bv", [H], F32, kind="ExternalInput")
        u = nc.dram_tensor("u", [S, H], F32, kind="ExternalOutput")
    else:
        u = nc.dram_tensor("u", [S, H], F32, kind="Internal")
    lo = nc.dram_tensor("l", [S], F32, kind="ExternalOutput")

    with tile.TileContext(nc) as tc:
        with (
            tc.tile_pool(name="small", bufs=1) as small,
            tc.tile_pool(name="p_qt", bufs=1) as p_qt,
            tc.tile_pool(name="p_kt", bufs=1) as p_kt,
            tc.tile_pool(name="p_v", bufs=1) as p_v,
            tc.tile_pool(name="p_x", bufs=1) as p_x,
            tc.tile_pool(name="p_w", bufs=3) as p_w,
            tc.tile_pool(name="p_e", bufs=1) as p_e,
            tc.tile_pool(name="p_us", bufs=2) as p_us,
            tc.tile_pool(name="ps", bufs=2, space="PSUM") as ps,
        ):
            bq_sb = small.tile([P, OT], F32)
            bk_sb = small.tile([P, OT], F32)
            bv_bc = small.tile([P, H], F32)
            ones_sb = small.tile([P, 8], BF16)
            l_sb = small.tile([P, IT], F32)

            qt_sb = p_qt.tile([P, OT, S], BF16)     # q^T: [o_in, o_tile, s]
            kt_sb = p_kt.tile([P, OT, SH], BF16)    # k^T: [o_in, o_tile, k]
            v_sb = p_v.tile([P, MT, H], BF16)       # V:   [k_in, k_tile, o]
            xth_sb = p_x.tile([P, HT, SH], BF16)
            xt_sb = p_x.tile([P, HT, S], BF16)
            wk_sb = p_w.tile([P, HT, H], BF16, tag="w")
            wv_sb = p_w.tile([P, HT, H], BF16, tag="w")
            wq_sb = p_w.tile([P, HT, H], BF16, tag="w")
            e_sb = p_e.tile([P, MT, S], BF16)       # E: [k_in, k_tile, q]

            nc.vector.memset(ones_sb[:], 1.0)

            def emit_inputs():
                # ---- input loads (consumption order: K, V, then Q) ----
                if not clone:
                    nc.sync.dma_start(bk_sb[:], bk.ap().rearrange("(t p) -> p t", p=P))
                    nc.sync.dma_start(bq_sb[:], bqs.ap().rearrange("(t p) -> p t", p=P))
                    bv_ap = bv.ap()
                    nc.gpsimd.dma_start(
                        out=bv_bc[:],
                        in_=bass.AP(tensor=bv_ap.tensor, offset=bv_ap.offset,
                                    ap=[[0, P], [1, H]]))
                    for j in range(HT):
                        nc.sync.dma_start(
                            wk_sb[:, j, :],
                            wkt.ap().rearrange("(j p) o -> p j o", p=P)[:, j, :])
                        nc.sync.dma_start(
                            xth_sb[:, j, :],
                            xth.ap().rearrange("(j p) s -> p j s", p=P)[:, j, :])
                    for j in range(HT):
                        nc.sync.dma_start(
                            wv_sb[:, j, :],
                            wvt.ap().rearrange("(j p) o -> p j o", p=P)[:, j, :])
                    for j in range(HT):
                        nc.sync.dma_start(
                            wq_sb[:, j, :],
                            wqt.ap().rearrange("(j p) o -> p j o", p=P)[:, j, :])
                        nc.sync.dma_start(
                            xt_sb[:, j, :],
                            xt.ap().rearrange("(j p) s -> p j s", p=P)[:, j, :])
                else:
                    nc.gpsimd.memset(bq_sb[:], 0.001)
                    nc.gpsimd.memset(bk_sb[:], 0.001)
                    nc.gpsimd.memset(bv_bc[:], 0.001)
                    for j in range(HT):
                        nc.gpsimd.memset(wk_sb[:, j, :], 0.01)
                        nc.gpsimd.memset(xth_sb[:, j, :], 0.01)
                    for j in range(HT):
                        nc.gpsimd.memset(wv_sb[:, j, :], 0.01)
                    for j in range(HT):
                        nc.gpsimd.memset(wq_sb[:, j, :], 0.01)
                        nc.gpsimd.memset(xt_sb[:, j, :], 0.01)

            def emit_compute():
                # ---- K projection ----
                for t in range(OT):
                    psk = ps.tile([P, QC, 512], F32, tag="ps", name="psk")
                    for j in range(HT):
                        for kc in range(KC):
                            nc.tensor.matmul(
                                psk[:, kc, :],
                                lhsT=wk_sb[:, j, t * P:(t + 1) * P],
                                rhs=xth_sb[:, j, kc * 512:(kc + 1) * 512],
                                start=(j == 0), stop=(j == HT - 1))
                    nc.scalar.activation(
                        kt_sb[:, t, :].rearrange("p (a b) -> p a b", b=512),
                        psk[:, :KC, :], Act.Identity, bias=bk_sb[:, t:t + 1])

                # ---- V projection ----
                for m in range(MT):
                    psv = ps.tile([P, QC, 512], F32, tag="ps", name="psv")
                    for j in range(HT):
                        for oc in range(OC):
                            nc.tensor.matmul(
                                psv[:, oc, :],
                                lhsT=xth_sb[:, j, m * P:(m + 1) * P],
                                rhs=wv_sb[:, j, oc * 512:(oc + 1) * 512],
                                start=(j == 0), stop=(j == HT - 1))
                    nc.vector.tensor_add(
                        v_sb[:, m, :].rearrange("p (a b) -> p a b", b=512),
                        psv[:, :OC, :],
                        bv_bc[:].rearrange("p (a b) -> p a b", b=512))

                # ---- Q projection (all queries) ----
                for t in range(OT):
                    psq = ps.tile([P, QC, 512], F32, tag="ps", name="psq")
                    for j in range(HT):
                        for qc in range(QC):
                            nc.tensor.matmul(
                                psq[:, qc, :],
                                lhsT=wq_sb[:, j, t * P:(t + 1) * P],
                                rhs=xt_sb[:, j, qc * 512:(qc + 1) * 512],
                                start=(j == 0), stop=(j == HT - 1))
                    nc.scalar.activation(
                        qt_sb[:, t, :].rearrange("p (a b) -> p a b", b=512),
                        psq[:], Act.Identity, bias=bq_sb[:, t:t + 1])

                # ---- scores^T + exp ----
                for m in range(MT):
                    pss = ps.tile([P, QC, 512], F32, tag="ps", name="pss")
                    for t in range(OT):
                        for qc in range(QC):
                            nc.tensor.matmul(
                                pss[:, qc, :],
                                lhsT=kt_sb[:, t, m * P:(m + 1) * P],
                                rhs=qt_sb[:, t, qc * 512:(qc + 1) * 512],
                                start=(t == 0), stop=(t == OT - 1))
                    nc.scalar.activation(
                        e_sb[:, m, :].rearrange("p (a b) -> p a b", b=512),
                        pss[:], Act.Exp)

                # ---- U = E.T @ V, l = E.T @ ones ----
                for i in range(IT):
                    pst = ps.tile([P, QC, 512], F32, tag="ps", name="pst")
                    psu_t = pst[:, 0:OC, :]
                    psl_t = pst[:, OC, 0:8]
                    for m in range(MT):
                        for oc in range(OC):
                            nc.tensor.matmul(
                                psu_t[:, oc, :],
                                lhsT=e_sb[:, m, i * P:(i + 1) * P],
                                rhs=v_sb[:, m, oc * 512:(oc + 1) * 512],
                                start=(m == 0), stop=(m == MT - 1))
                        nc.tensor.matmul(
                            psl_t,
                            lhsT=e_sb[:, m, i * P:(i + 1) * P],
                            rhs=ones_sb[:],
                            start=(m == 0), stop=(m == MT - 1))
                    u_t = p_us.tile([P, OC, 512], F32, tag="us", name="u_t")
                    nc.vector.tensor_copy(u_t[:], psu_t[:])
                    nc.vector.tensor_copy(l_sb[:, i:i + 1], psl_t[:, 0:1])
                    nc.sync.dma_start(
                        u.ap()[i * P:(i + 1) * P, :].rearrange(
                            "p (a b) -> p a b", b=512),
                        u_t[:])
                nc.sync.dma_start(
                    lo.ap().rearrange("(i p) -> p i", p=P), l_sb[:])

            if loop_n is not None:
                emit_inputs()
                with tc.For_i(0, loop_n, 1):
                    emit_compute()
            else:
                emit_inputs()
                emit_compute()

    nc.compile()
    return nc


_NC_CACHE = None


def _get_nc():
    global _NC_CACHE
    if _NC_CACHE is None:
        _NC_CACHE = build_nc()
    return _NC_CACHE


def make_in_maps(hidden_states, Wq, bq, Wk, bk, Wv, bv):
    bf = ml_dtypes.bfloat16
    scale = 1.0 / np.sqrt(np.float32(H))
    wqt = np.ascontiguousarray(Wq.T * scale).astype(bf)
    wkt = np.ascontiguousarray(Wk.T).astype(bf)
    wvt = np.ascontiguousarray(Wv.T).astype(bf)
    bqs = (bq * scale).astype(np.float32)
    bk32 = bk.astype(np.float32)
    bv32 = bv.astype(np.float32)
    in_maps = []
    for c in range(8):
        b, half = divmod(c, 2)
        xtb = np.ascontiguousarray(hidden_states[b].T).astype(bf)
        in_maps.append({
            "xt": xtb,
            "xth": np.ascontiguousarray(xtb[:, half * SH:(half + 1) * SH]),
            "wqt": wqt, "wkt": wkt, "wvt": wvt,
            "bqs": bqs, "bk": bk32, "bv": bv32,
        })
    return in_maps


def combine(results):
    out = np.empty((B, S, H), np.float32)
    for b in range(B):
        r0, r1 = results[2 * b], results[2 * b + 1]
        usum = r0["u"] + r1["u"]
        lsum = r0["l"] + r1["l"]
        out[b] = usum / lsum[:, None]
    return out


def kernel(hidden_states, Wq, bq, Wk, bk, Wv, bv):
    nc = _get_nc()
    in_maps = make_in_maps(
        np.asarray(hidden_states, np.float32),
        np.asarray(Wq, np.float32), np.asarray(bq, np.float32),
        np.asarray(Wk, np.float32), np.asarray(bk, np.float32),
        np.asarray(Wv, np.float32), np.asarray(bv, np.float32),
    )
    res = run_bass_kernel_spmd(nc, in_maps, core_ids=list(range(8)))
    return combine(res.results)


# revision 21
# speedup vs baseline: 1.0521x; 1.0521x over previous
"""Trainium2 Bass kernel for single-head attention layer.

Problem: B=4, S=2048, H=1024 fp32.
  q = x @ Wq.T + bq ; k = x @ Wk.T + bk ; v = x @ Wv.T + bv
  out = softmax(q @ k.T / sqrt(H)) @ v

Sharding (8 cores): core c handles batch b=c//2 and key-half half=c%2.
Each core computes (for its batch):
  qT   [H, 2048]  = scaled Q projection for all queries (duplicated per pair)
  kT   [H, 1024]  = keys for its half
  V    [1024, H]  = values for its half
  E    [1024k, 2048q] = exp(scores^T)  (no max subtraction -- scores ~ N(0,1),
                        exp is safe in fp32)
  U    [2048, H]  = E.T @ V   (unnormalized output, fp32 PSUM accum)
  l    [2048]     = per-query sum of E (ones-column matmul)
Host combines: out[b] = (U0+U1) / (l0+l1)[:, None].

All host-side prep (transposes, 1/sqrt(H) folding into Wq/bq, bf16 casts)
is free -- only NEFF execution time counts.
"""

import numpy as np
import ml_dtypes

import concourse.bass as bass
import concourse.mybir as mybir
import concourse.tile as tile
from concourse import bacc
from concourse.bass_utils import run_bass_kernel_spmd

BF16 = mybir.dt.bfloat16
F32 = mybir.dt.float32

B, S, H = 4, 2048, 1024
SH = S // 2          # per-core key half
P = 128
HT = H // P          # 8 h-tiles (contraction for projections)
OT = H // P          # 8 o-tiles
QC = S // 512        # 4 q-chunks of 512
KC = SH // 512       # 2 k-chunks of 512
OC = H // 512        # 2 o-chunks of 512
MT = SH // P         # 8 key tiles (my half)
IT = S // P          # 16 query tiles

Act = mybir.ActivationFunctionType


def build_nc(clone=False, loop_n=None):
    """clone=True: no external inputs (memset instead), u internal -- for timing.
    loop_n: wrap the body in a hardware For_i loop (timing amplification)."""
    nc = bacc.Bacc("TRN2", target_bir_lowering=False, debug=False, num_devices=1)

    if not clone:
        xt = nc.dram_tensor("xt", [H, S], BF16, kind="ExternalInput")   # x[b].T, my key-half columns first
        wqt = nc.dram_tensor("wqt", [H, H], BF16, kind="ExternalInput")  # Wq.T/32
        wkt = nc.dram_tensor("wkt", [H, H], BF16, kind="ExternalInput")  # Wk.T
        wvt = nc.dram_tensor("wvt", [H, H], BF16, kind="ExternalInput")  # Wv.T
        bqs = nc.dram_tensor("bqs", [H], F32, kind="ExternalInput")      # bq/32
        bk = nc.dram_tensor("bk", [H], F32, kind="ExternalInput")
        bv = nc.dram_tensor("bv", [H], F32, kind="ExternalInput")
        u = nc.dram_tensor("u", [S, H], F32, kind="ExternalOutput")
    else:
        u = nc.dram_tensor("u", [S, H], F32, kind="Internal")
    lo = nc.dram_tensor("l", [S], F32, kind="ExternalOutput")

    with tile.TileContext(nc) as tc:
        with (
            tc.tile_pool(name="small", bufs=1) as small,
            tc.tile_pool(name="p_qt", bufs=1) as p_qt,
            tc.tile_pool(name="p_kt", bufs=1) as p_kt,
            tc.tile_pool(name="p_v", bufs=1) as p_v,
            tc.tile_pool(name="p_x", bufs=1) as p_x,
            tc.tile_pool(name="p_w", bufs=3) as p_w,
            tc.tile_pool(name="p_e", bufs=1) as p_e,
            tc.tile_pool(name="p_us", bufs=3) as p_us,
            tc.tile_pool(name="ps", bufs=2, space="PSUM") as ps,
        ):
            bq_sb = small.tile([P, OT], F32)
            bk_sb = small.tile([P, OT], F32)
            bv_bc = small.tile([P, H], F32)
            ones_sb = small.tile([P, 8], BF16)
            l_sb = small.tile([P, IT], F32)

            qt_sb = p_qt.tile([P, OT, S], BF16)     # q^T: [o_in, o_tile, s]
            kt_sb = p_kt.tile([P, OT, SH], BF16)    # k^T: [o_in, o_tile, k]
            v_sb = p_v.tile([P, MT, H], BF16)       # V:   [k_in, k_tile, o]
            xt_sb = p_x.tile([P, HT, S], BF16)
            wk_sb = p_w.tile([P, HT, H], BF16, tag="w")
            wv_sb = p_w.tile([P, HT, H], BF16, tag="w")
            wq_sb = p_w.tile([P, HT, H], BF16, tag="w")
            e_sb = p_e.tile([P, MT, S], BF16)       # E: [k_in, k_tile, q]

            nc.vector.memset(ones_sb[:], 1.0)

            def emit_inputs():
                # ---- input loads (consumption order: K, V, then Q) ----
                if not clone:
                    nc.sync.dma_start(bk_sb[:], bk.ap().rearrange("(t p) -> p t", p=P))
                    nc.sync.dma_start(bq_sb[:], bqs.ap().rearrange("(t p) -> p t", p=P))
                    bv_ap = bv.ap()
                    nc.gpsimd.dma_start(
                        out=bv_bc[:],
                        in_=bass.AP(tensor=bv_ap.tensor, offset=bv_ap.offset,
                                    ap=[[0, P], [1, H]]))
                    for j in range(HT):
                        nc.sync.dma_start(
                            wk_sb[:, j, :],
                            wkt.ap().rearrange("(j p) o -> p j o", p=P)[:, j, :])
                        nc.sync.dma_start(
                            xt_sb[:, j, 0:SH],
                            xt.ap().rearrange("(j p) s -> p j s", p=P)[:, j, 0:SH])
                    for j in range(HT):
                        nc.sync.dma_start(
                            wv_sb[:, j, :],
                            wvt.ap().rearrange("(j p) o -> p j o", p=P)[:, j, :])
                    for j in range(HT):
                        nc.sync.dma_start(
                            wq_sb[:, j, :],
                            wqt.ap().rearrange("(j p) o -> p j o", p=P)[:, j, :])
                        nc.sync.dma_start(
                            xt_sb[:, j, SH:S],
                            xt.ap().rearrange("(j p) s -> p j s", p=P)[:, j, SH:S])
                else:
                    nc.gpsimd.memset(bq_sb[:], 0.001)
                    nc.gpsimd.memset(bk_sb[:], 0.001)
                    nc.gpsimd.memset(bv_bc[:], 0.001)
                    for j in range(HT):
                        nc.gpsimd.memset(wk_sb[:, j, :], 0.01)
                    for j in range(HT):
                        nc.gpsimd.memset(wv_sb[:, j, :], 0.01)
                    for j in range(HT):
                        nc.gpsimd.memset(wq_sb[:, j, :], 0.01)
                        nc.gpsimd.memset(xt_sb[:, j, :], 0.01)

            def emit_compute():
                # ---- K projection ----
                for t in range(OT):
                    psk = ps.tile([P, QC, 512], F32, tag="ps", name="psk")
                    for j in range(HT):
                        for kc in range(KC):
                            nc.tensor.matmul(
                                psk[:, kc, :],
                                lhsT=wk_sb[:, j, t * P:(t + 1) * P],
                                rhs=xt_sb[:, j, kc * 512:(kc + 1) * 512],
                                start=(j == 0), stop=(j == HT - 1))
                    nc.scalar.activation(
                        kt_sb[:, t, :].rearrange("p (a b) -> p a b", b=512),
                        psk[:, :KC, :], Act.Identity, bias=bk_sb[:, t:t + 1])

                # ---- V projection ----
                for m in range(MT):
                    psv = ps.tile([P, QC, 512], F32, tag="ps", name="psv")
                    for j in range(HT):
                        for oc in range(OC):
                            nc.tensor.matmul(
                                psv[:, oc, :],
                                lhsT=xt_sb[:, j, m * P:(m + 1) * P],
                                rhs=wv_sb[:, j, oc * 512:(oc + 1) * 512],
                                start=(j == 0), stop=(j == HT - 1))
                    nc.vector.tensor_add(
                        v_sb[:, m, :].rearrange("p (a b) -> p a b", b=512),
                        psv[:, :OC, :],
                        bv_bc[:].rearrange("p (a b) -> p a b", b=512))

                # ---- Q projection (all queries) ----
                for t in range(OT):
                    psq = ps.tile([P, QC, 512], F32, tag="ps", name="psq")
                    for j in range(HT):
                        for qc in range(QC):
                            nc.tensor.matmul(
                                psq[:, qc, :],
                                lhsT=wq_sb[:, j, t * P:(t + 1) * P],
                                rhs=xt_sb[:, j, qc * 512:(qc + 1) * 512],
                                start=(j == 0), stop=(j == HT - 1))
                    nc.scalar.activation(
                        qt_sb[:, t, :].rearrange("p (a b) -> p a b", b=512),
                        psq[:], Act.Identity, bias=bq_sb[:, t:t + 1])

                # ---- scores^T + exp ----
                for m in range(MT):
                    pss = ps.tile([P, QC, 512], F32, tag="ps", name="pss")
                    for t in range(OT):
                        for qc in range(QC):
                            nc.tensor.matmul(
                                pss[:, qc, :],
                                lhsT=kt_sb[:, t, m * P:(m + 1) * P],
                                rhs=qt_sb[:, t, qc * 512:(qc + 1) * 512],
                                start=(t == 0), stop=(t == OT - 1))
                    nc.scalar.activation(
                        e_sb[:, m, :].rearrange("p (a b) -> p a b", b=512),
                        pss[:], Act.Exp)

                # ---- U = E.T @ V, l = E.T @ ones ----
                for i in range(IT):
                    pst = ps.tile([P, QC, 512], F32, tag="ps", name="pst")
                    psu_t = pst[:, 0:OC, :]
                    psl_t = pst[:, OC, 0:8]
                    for m in range(MT):
                        for oc in range(OC):
                            nc.tensor.matmul(
                                psu_t[:, oc, :],
                                lhsT=e_sb[:, m, i * P:(i + 1) * P],
                                rhs=v_sb[:, m, oc * 512:(oc + 1) * 512],
                                start=(m == 0), stop=(m == MT - 1))
                        nc.tensor.matmul(
                            psl_t,
                            lhsT=e_sb[:, m, i * P:(i + 1) * P],
                            rhs=ones_sb[:],
                            start=(m == 0), stop=(m == MT - 1))
                    u_t = p_us.tile([P, OC, 512], F32, tag="us", name="u_t")
                    nc.vector.tensor_copy(u_t[:], psu_t[:])
                    nc.vector.tensor_copy(l_sb[:, i:i + 1], psl_t[:, 0:1])
                    nc.sync.dma_start(
                        u.ap()[i * P:(i + 1) * P, :].rearrange(
                            "p (a b) -> p a b", b=512),
                        u_t[:])
                nc.sync.dma_start(
                    lo.ap().rearrange("(i p) -> p i", p=P), l_sb[:])

            if loop_n is not None:
                emit_inputs()
                with tc.For_i(0, loop_n, 1):
                    emit_compute()
            else:
                emit_inputs()
                emit_compute()

    nc.compile()
    return nc


_NC_CACHE = None


def _get_nc():
    global _NC_CACHE
    if _NC_CACHE is None:
        _NC_CACHE = build_nc()
    return _NC_CACHE


def make_in_maps(hidden_states, Wq, bq, Wk, bk, Wv, bv):
    bf = ml_dtypes.bfloat16
    scale = 1.0 / np.sqrt(np.float32(H))
    wqt = np.ascontiguousarray(Wq.T * scale).astype(bf)
    wkt = np.ascontiguousarray(Wk.T).astype(bf)
    wvt = np.ascontiguousarray(Wv.T).astype(bf)
    bqs = (bq * scale).astype(np.float32)
    bk32 = bk.astype(np.float32)
    bv32 = bv.astype(np.float32)
    in_maps = []
    for c in range(8):
        b, half = divmod(c, 2)
        xtb = np.asarray(hidden_states[b].T).astype(bf)
        if half == 1:
            xtb = np.concatenate([xtb[:, SH:], xtb[:, :SH]], axis=1)
        in_maps.append({
            "xt": np.ascontiguousarray(xtb),
            "wqt": wqt, "wkt": wkt, "wvt": wvt,
            "bqs": bqs, "bk": bk32, "bv": bv32,
        })
    return in_maps


def combine(results):
    out = np.empty((B, S, H), np.float32)
    for b in range(B):
        r0, r1 = results[2 * b], results[2 * b + 1]
        # core 2b+1 ran with its key-half first, so its query axis is
        # half-swapped; rotate its U rows / l entries back
        u1 = np.concatenate([r1["u"][SH:], r1["u"][:SH]], axis=0)
        l1 = np.concatenate([r1["l"][SH:], r1["l"][:SH]])
        usum = r0["u"] + u1
        lsum = r0["l"] + l1
        out[b] = usum / lsum[:, None]
    return out


def kernel(hidden_states, Wq, bq, Wk, bk, Wv, bv):
    nc = _get_nc()
    in_maps = make_in_maps(
        np.asarray(hidden_states, np.float32),
        np.asarray(Wq, np.float32), np.asarray(bq, np.float32),
        np.asarray(Wk, np.float32), np.asarray(bk, np.float32),
        np.asarray(Wv, np.float32), np.asarray(bv, np.float32),
    )
    res = run_bass_kernel_spmd(nc, in_maps, core_ids=list(range(8)))
    return combine(res.results)
